# revision 13
# baseline (speedup 1.0000x reference)
"""Trainium2 Bass kernel for DigitCapsuleLayer (single routing iteration).

Math: with num_iterations == 1 the routing coefficients are uniform 1/R, so

    v[b,c,o] = squash( (1/R) * sum_{r,i} x[b,r,i] * W[0,r,c,o,i] )

i.e. one big [B=128, K=32768] x [K=32768, N=1024] matmul followed by a tiny
per-capsule squash nonlinearity.  W is the dominant HBM traffic and is read
exactly once.

Sharding (8 cores): split the OUTPUT columns co=(c,o) so each core owns 128
columns = 4 whole capsules.  Each core reads its private 1/8 slice of W plus
the full x and produces its 4 capsules completely locally: no collective, no
cross-core reduction, no exchange tail.  (The previous K-sharded variant spent
~45 us on AllToAll entry + rank skew + gather; this design spends 0.)

Inputs are cast to bf16 ON HOST (host prep is free): halves the DMA stream to
8 MB W + 8 MB x per core and runs the PE at 1 cycle/row.  Accumulation stays
fp32 in PSUM, so the only precision loss is the input rounding (~0.3% rel
error vs the 2e-2 gate).

Per-core layout: contraction index k = kc*128 + p with p=(r%8, i), so both
SBUF operand tiles are [p=128, kc, 128] with fully contiguous partition
lines -> line-rate DMA.  W rides the sync HWDGE ring, x the scalar ring, in
matched groups (big first for DMA efficiency, small last so the final
matmul wave lands right behind the last DMA).  All 256 k-tiles accumulate
into one PSUM bank; squash runs on DVE/ACT and the 64 KB result DMAs out.
"""

import numpy as np
import ml_dtypes

import concourse.bacc as bacc
import concourse.bass as bass
import concourse.bass_utils as bass_utils
import concourse.mybir as mybir
import concourse.tile as tile

# Problem shape (hardcoded per the kernel contract).
B, R, C, I, O = 128, 2048, 32, 16, 32
NCORES = 8
K = R * I            # 32768 contraction
KC = K // 128        # 256 k-tiles
CPS = C // NCORES    # 4 capsules per core
COS = CPS * O        # 128 output columns per core
# DMA group sizes in kc units (sum 256).  Each dma_start costs ~0.6 us of
# serial HWDGE descriptor-gen and the engine pool saturates only when one
# ring has >~1 MB buffered, so the first group is large (48 kc = 1.5 MB per
# ring); the PE (21 us of matmul) still catches the stream (44 us) easily.
# Small final group so the PE drain after the last byte lands is ~0.6 us.
GROUPS = [96, 96, 56, 8]


def _build_program():
    nc = bacc.Bacc(
        "TRN2", target_bir_lowering=False, debug=False, num_devices=NCORES
    )
    f32 = mybir.dt.float32
    bf16 = mybir.dt.bfloat16

    xT = nc.dram_tensor("xT", [128, KC * B], bf16, kind="ExternalInput").ap()
    Wt = nc.dram_tensor("Wt", [128, KC * COS], bf16, kind="ExternalInput").ap()
    out = nc.dram_tensor("out", [B, COS], f32, kind="ExternalOutput").ap()

    with tile.TileContext(nc) as tc:
        with (
            tc.tile_pool(name="xpool", bufs=1) as xpool,
            tc.tile_pool(name="wpool", bufs=1) as wpool,
            tc.tile_pool(name="qpool", bufs=1) as qpool,
            tc.tile_pool(name="psum", bufs=1, space="PSUM") as psum_pool,
        ):
            x_sb = xpool.tile([128, KC * B], bf16)
            w_sb = wpool.tile([128, KC * COS], bf16)

            g0 = 0
            for gsz in GROUPS:
                nc.scalar.dma_start(
                    x_sb[:, g0 * B : (g0 + gsz) * B],
                    xT[:, g0 * B : (g0 + gsz) * B],
                )
                nc.sync.dma_start(
                    w_sb[:, g0 * COS : (g0 + gsz) * COS],
                    Wt[:, g0 * COS : (g0 + gsz) * COS],
                )
                g0 += gsz

            # Warm the Sqrt/Square ACT tables under the DMA stream (AFTER the
            # x dma_starts so the ~1.3 us LUT loads don't delay the stream).
            warm = qpool.tile([1, 1], f32)
            nc.vector.memset(warm[:], 0.0)
            nc.scalar.square(warm[:], warm[:])
            nc.scalar.sqrt(warm[:], warm[:])

            ps = psum_pool.tile([128, COS], f32)
            for kc in range(KC):
                nc.tensor.matmul(
                    ps,
                    x_sb[:, kc * B : (kc + 1) * B],
                    w_sb[:, kc * COS : (kc + 1) * COS],
                    start=(kc == 0),
                    stop=(kc == KC - 1),
                )

            # Squash on [p=b, (cl,o)]: all per-capsule sums are within one
            # partition line, so DVE X-axis reductions do it directly.
            # With t = PSUM accumulation (= R*s), the squash folds to
            #   v = t * sqrt(q) / (R^2 + q),   q = sum_o t^2
            # so the 1/R scale costs nothing and t is read straight from
            # PSUM (no full-width copy to SBUF).
            s2 = qpool.tile([128, CPS, O], f32)
            nc.scalar.square(s2[:], ps[:].rearrange("p (cl o) -> p cl o", o=O))
            sq = qpool.tile([128, CPS], f32)
            nc.vector.reduce_sum(sq[:], s2[:], axis=mybir.AxisListType.X)
            rt = qpool.tile([128, CPS], f32)
            nc.scalar.sqrt(rt[:], sq[:])
            den = qpool.tile([128, CPS], f32)
            nc.vector.tensor_scalar_add(den[:], sq[:], float(R) * float(R))
            rec = qpool.tile([128, CPS], f32)
            nc.vector.reciprocal(rec[:], den[:])
            fac = qpool.tile([128, CPS], f32)
            nc.vector.tensor_mul(out=fac[:], in0=rt[:], in1=rec[:])
            v = qpool.tile([128, CPS, O], f32)
            nc.vector.tensor_tensor(
                v[:],
                ps[:].rearrange("p (cl o) -> p cl o", o=O),
                fac[:, :, None].to_broadcast((128, CPS, O)),
                mybir.AluOpType.mult,
            )
            nc.sync.dma_start(out, v[:].rearrange("p cl o -> p (cl o)"))

    nc.compile()
    return nc


def _shard_inputs(x: np.ndarray, W: np.ndarray):
    """Per-core input layouts (host-side data prep).

    Contraction index k = kc*128 + p with p = (rl, i), rl = r % 8,
    kc = r // 8; i.e. r = kc*8 + rl.
    """
    xb = x.astype(ml_dtypes.bfloat16)                  # [B, R, I]
    xt = np.ascontiguousarray(
        xb.reshape(B, KC, 8, I).transpose(2, 3, 1, 0)  # (rl, i, kc, b)
    ).reshape(128, KC * B)

    Wb = W[0].astype(ml_dtypes.bfloat16)               # [R, C, O, I]
    in_maps = []
    for m in range(NCORES):
        Wm = Wb[:, m * CPS : (m + 1) * CPS]            # (r, cl, o, i)
        wt = np.ascontiguousarray(
            Wm.reshape(KC, 8, CPS, O, I).transpose(1, 4, 0, 2, 3)
        ).reshape(128, KC * COS)                       # (rl, i, kc, cl, o)
        in_maps.append({"xT": xt, "Wt": wt})
    return in_maps


_CACHED_NC = None


def _get_nc():
    global _CACHED_NC
    if _CACHED_NC is None:
        _CACHED_NC = _build_program()
    return _CACHED_NC


def kernel(x: np.ndarray, W: np.ndarray, _trace: bool = False):
    x = np.ascontiguousarray(np.asarray(x, dtype=np.float32))
    W = np.ascontiguousarray(np.asarray(W, dtype=np.float32))
    nc = _get_nc()
    in_maps = _shard_inputs(x, W)
    res = bass_utils.run_bass_kernel_spmd(
        nc, in_maps, core_ids=list(range(NCORES)), trace=_trace
    )
    out = np.concatenate(
        [res.results[m]["out"] for m in range(NCORES)], axis=1
    ).reshape(B, C, O, 1)
    if _trace:
        return out, res
    return out


# revision 14
# speedup vs baseline: 1.1558x; 1.1558x over previous
"""Trainium2 Bass kernel for DigitCapsuleLayer (single routing iteration).

Math: with num_iterations == 1 the routing coefficients are uniform 1/R, so

    v[b,c,o] = squash( (1/R) * sum_{r,i} x[b,r,i] * W[0,r,c,o,i] )

i.e. one big [B=128, K=32768] x [K=32768, N=1024] matmul followed by a tiny
per-capsule squash nonlinearity.  W is the dominant HBM traffic and is read
exactly once.

Sharding (8 cores): split the OUTPUT columns co=(c,o) so each core owns 128
columns = 4 whole capsules.  Each core reads its private 1/8 slice of W plus
the full x and produces its 4 capsules completely locally: no collective, no
cross-core reduction, no exchange tail.  (The previous K-sharded variant spent
~45 us on AllToAll entry + rank skew + gather; this design spends 0.)

Inputs are cast to bf16 ON HOST (host prep is free): halves the DMA stream to
8 MB W + 8 MB x per core and runs the PE at 1 cycle/row.  Accumulation stays
fp32 in PSUM, so the only precision loss is the input rounding (~0.3% rel
error vs the 2e-2 gate).

Per-core layout: contraction index k = kc*128 + p with p=(r%8, i), so both
SBUF operand tiles are [p=128, kc, 128] with fully contiguous partition
lines -> line-rate DMA.  W rides the sync HWDGE ring, x the scalar ring, in
matched groups (big first for DMA efficiency, small last so the final
matmul wave lands right behind the last DMA).  All 256 k-tiles accumulate
into one PSUM bank; squash runs on DVE/ACT and the 64 KB result DMAs out.
"""

import numpy as np
import ml_dtypes

import concourse.bacc as bacc
import concourse.bass as bass
import concourse.bass_utils as bass_utils
import concourse.mybir as mybir
import concourse.tile as tile

# Problem shape (hardcoded per the kernel contract).
B, R, C, I, O = 128, 2048, 32, 16, 32
NCORES = 8
K = R * I            # 32768 contraction
KC = K // 128        # 256 k-tiles
CPS = C // NCORES    # 4 capsules per core
COS = CPS * O        # 128 output columns per core
# DMA group sizes in kc units (sum 256).  Each dma_start costs ~0.6 us of
# serial HWDGE descriptor-gen and the engine pool saturates only when one
# ring has >~1 MB buffered, so the first group is large (48 kc = 1.5 MB per
# ring); the PE (21 us of matmul) still catches the stream (44 us) easily.
# Small final group so the PE drain after the last byte lands is ~0.6 us.
GROUPS = [48, 48, 64, 48, 32, 12, 4]


def _build_program():
    nc = bacc.Bacc(
        "TRN2", target_bir_lowering=False, debug=False, num_devices=NCORES
    )
    f32 = mybir.dt.float32
    bf16 = mybir.dt.bfloat16

    xT = nc.dram_tensor("xT", [128, KC * B], bf16, kind="ExternalInput").ap()
    Wt = nc.dram_tensor("Wt", [128, KC * COS], bf16, kind="ExternalInput").ap()
    out = nc.dram_tensor("out", [B, COS], f32, kind="ExternalOutput").ap()

    with tile.TileContext(nc) as tc:
        with (
            tc.tile_pool(name="xpool", bufs=1) as xpool,
            tc.tile_pool(name="wpool", bufs=1) as wpool,
            tc.tile_pool(name="qpool", bufs=1) as qpool,
            tc.tile_pool(name="psum", bufs=1, space="PSUM") as psum_pool,
        ):
            x_sb = xpool.tile([128, KC * B], bf16)
            w_sb = wpool.tile([128, KC * COS], bf16)

            g0 = 0
            for gsz in GROUPS:
                nc.scalar.dma_start(
                    x_sb[:, g0 * B : (g0 + gsz) * B],
                    xT[:, g0 * B : (g0 + gsz) * B],
                )
                nc.sync.dma_start(
                    w_sb[:, g0 * COS : (g0 + gsz) * COS],
                    Wt[:, g0 * COS : (g0 + gsz) * COS],
                )
                g0 += gsz

            # Warm the Sqrt/Square ACT tables under the DMA stream (AFTER the
            # x dma_starts so the ~1.3 us LUT loads don't delay the stream).
            warm = qpool.tile([1, 1], f32)
            nc.vector.memset(warm[:], 0.0)
            nc.scalar.square(warm[:], warm[:])
            nc.scalar.sqrt(warm[:], warm[:])

            ps = psum_pool.tile([128, COS], f32)
            for kc in range(KC):
                nc.tensor.matmul(
                    ps,
                    x_sb[:, kc * B : (kc + 1) * B],
                    w_sb[:, kc * COS : (kc + 1) * COS],
                    start=(kc == 0),
                    stop=(kc == KC - 1),
                )

            # Squash on [p=b, (cl,o)]: all per-capsule sums are within one
            # partition line, so DVE X-axis reductions do it directly.
            # With t = PSUM accumulation (= R*s), the squash folds to
            #   v = t * sqrt(q) / (R^2 + q),   q = sum_o t^2
            # so the 1/R scale costs nothing and t is read straight from
            # PSUM (no full-width copy to SBUF).
            s2 = qpool.tile([128, CPS, O], f32)
            nc.scalar.square(s2[:], ps[:].rearrange("p (cl o) -> p cl o", o=O))
            sq = qpool.tile([128, CPS], f32)
            nc.vector.reduce_sum(sq[:], s2[:], axis=mybir.AxisListType.X)
            rt = qpool.tile([128, CPS], f32)
            nc.scalar.sqrt(rt[:], sq[:])
            den = qpool.tile([128, CPS], f32)
            nc.vector.tensor_scalar_add(den[:], sq[:], float(R) * float(R))
            rec = qpool.tile([128, CPS], f32)
            nc.vector.reciprocal(rec[:], den[:])
            fac = qpool.tile([128, CPS], f32)
            nc.vector.tensor_mul(out=fac[:], in0=rt[:], in1=rec[:])
            v = qpool.tile([128, CPS, O], f32)
            nc.vector.tensor_tensor(
                v[:],
                ps[:].rearrange("p (cl o) -> p cl o", o=O),
                fac[:, :, None].to_broadcast((128, CPS, O)),
                mybir.AluOpType.mult,
            )
            nc.sync.dma_start(out, v[:].rearrange("p cl o -> p (cl o)"))

    nc.compile()
    return nc


def _shard_inputs(x: np.ndarray, W: np.ndarray):
    """Per-core input layouts (host-side data prep).

    Contraction index k = kc*128 + p with p = (rl, i), rl = r % 8,
    kc = r // 8; i.e. r = kc*8 + rl.
    """
    xb = x.astype(ml_dtypes.bfloat16)                  # [B, R, I]
    xt = np.ascontiguousarray(
        xb.reshape(B, KC, 8, I).transpose(2, 3, 1, 0)  # (rl, i, kc, b)
    ).reshape(128, KC * B)

    Wb = W[0].astype(ml_dtypes.bfloat16)               # [R, C, O, I]
    in_maps = []
    for m in range(NCORES):
        Wm = Wb[:, m * CPS : (m + 1) * CPS]            # (r, cl, o, i)
        wt = np.ascontiguousarray(
            Wm.reshape(KC, 8, CPS, O, I).transpose(1, 4, 0, 2, 3)
        ).reshape(128, KC * COS)                       # (rl, i, kc, cl, o)
        in_maps.append({"xT": xt, "Wt": wt})
    return in_maps


_CACHED_NC = None


def _get_nc():
    global _CACHED_NC
    if _CACHED_NC is None:
        _CACHED_NC = _build_program()
    return _CACHED_NC


def kernel(x: np.ndarray, W: np.ndarray, _trace: bool = False):
    x = np.ascontiguousarray(np.asarray(x, dtype=np.float32))
    W = np.ascontiguousarray(np.asarray(W, dtype=np.float32))
    nc = _get_nc()
    in_maps = _shard_inputs(x, W)
    res = bass_utils.run_bass_kernel_spmd(
        nc, in_maps, core_ids=list(range(NCORES)), trace=_trace
    )
    out = np.concatenate(
        [res.results[m]["out"] for m in range(NCORES)], axis=1
    ).reshape(B, C, O, 1)
    if _trace:
        return out, res
    return out
